# revision 2
# baseline (speedup 1.0000x reference)
import numpy as np
import jax
import jax.numpy as jnp
from jax.sharding import PartitionSpec as P

N, E, G, H, NF = 50000, 500000, 128, 256, 64
M = 8            # cores
NS = N // M      # node shard = 6250
LN_EPS = 1e-5

_cache = {}


def _layernorm(x, gamma, beta):
    mu = jnp.mean(x, axis=-1, keepdims=True)
    var = jnp.mean(jnp.square(x - mu), axis=-1, keepdims=True)
    return (x - mu) * jax.lax.rsqrt(var + LN_EPS) * gamma + beta


def _shard_fn(h, ei0, ei1, e2g, fd, lat9, ln_gamma, ln_beta,
              eW1, eb1, eW2, eb2, nW1, nb1, nW2, nb2):
    # h [N,H] replicated; ei [2,Eloc]; e2g [Eloc]; fd [Eloc,3]; lat9 [G,9]
    h_ln = _layernorm(h, ln_gamma, ln_beta)
    hi = h_ln[ei0]                        # [Eloc,H]
    hj = h_ln[ei1]
    lat_e = lat9[e2g]                     # [Eloc,9]
    freqs = 2.0 * np.pi * jnp.arange(NF, dtype=fd.dtype)
    emb = (fd[:, :, None] * freqs[None, None, :]).reshape(-1, 3 * NF)
    fe = jnp.concatenate([jnp.sin(emb), jnp.cos(emb)], axis=-1)  # [Eloc,384]
    e = jnp.concatenate([hi, hj, lat_e, fe], axis=1)             # [Eloc,905]
    e = jax.nn.silu(e @ eW1 + eb1)
    e = jax.nn.silu(e @ eW2 + eb2)                               # [Eloc,H]
    seg = ei0
    s = jax.ops.segment_sum(e, seg, num_segments=N)              # [N,H]
    c = jax.ops.segment_sum(jnp.ones((e.shape[0],), e.dtype), seg,
                            num_segments=N)                      # [N]
    s = jax.lax.psum_scatter(s, 'x', scatter_dimension=0, tiled=True)  # [NS,H]
    c = jax.lax.psum_scatter(c, 'x', scatter_dimension=0, tiled=True)  # [NS]
    agg = s / jnp.maximum(c, 1.0)[:, None]
    i = jax.lax.axis_index('x')
    h_ln_sh = jax.lax.dynamic_slice_in_dim(h_ln, i * NS, NS)
    h_sh = jax.lax.dynamic_slice_in_dim(h, i * NS, NS)
    out = jnp.concatenate([h_ln_sh, agg], axis=1)                # [NS,2H]
    out = jax.nn.silu(out @ nW1 + nb1)
    out = jax.nn.silu(out @ nW2 + nb2)
    return h_sh + out                                            # [NS,H]


def _get_jit():
    if 'fn' in _cache:
        return _cache['fn'], _cache['mesh']
    mesh = jax.make_mesh((M,), ('x',))
    rep = P()
    fn = jax.jit(jax.shard_map(
        _shard_fn, mesh=mesh,
        in_specs=(rep, P('x'), P('x'), P('x'), P('x', None), rep,
                  rep, rep, rep, rep, rep, rep, rep, rep, rep, rep),
        out_specs=P('x', None)))
    _cache['fn'] = fn
    _cache['mesh'] = mesh
    return fn, mesh


def kernel(h, frac_coords, lattices, edge_index, edge2graph, frac_diff,
           ln_gamma, ln_beta, eW1, eb1, eW2, eb2, nW1, nb1, nW2, nb2):
    fn, mesh = _get_jit()
    lat = np.asarray(lattices, np.float32)
    lat9 = np.einsum('gij,gkj->gik', lat, lat).reshape(G, 9)
    ei = np.asarray(edge_index, np.int32)
    ei0 = np.ascontiguousarray(ei[0]); ei1 = np.ascontiguousarray(ei[1])
    e2g = np.asarray(edge2graph, np.int32)
    args = (np.asarray(h, np.float32), ei0, ei1, e2g,
            np.asarray(frac_diff, np.float32), lat9.astype(np.float32),
            np.asarray(ln_gamma, np.float32), np.asarray(ln_beta, np.float32),
            np.asarray(eW1, np.float32), np.asarray(eb1, np.float32),
            np.asarray(eW2, np.float32), np.asarray(eb2, np.float32),
            np.asarray(nW1, np.float32), np.asarray(nb1, np.float32),
            np.asarray(nW2, np.float32), np.asarray(nb2, np.float32))
    out = fn(*args)
    return np.asarray(jax.device_get(out), np.float32)


# revision 4
# speedup vs baseline: 1.8999x; 1.8999x over previous
import numpy as np
import jax
import jax.numpy as jnp
from jax.sharding import PartitionSpec as P

N, E, G, H, NF = 50000, 500000, 128, 256, 64
M = 8            # cores
NS = N // M      # node shard = 6250
LN_EPS = 1e-5

_cache = {}
_timing = {}


def _layernorm(x, gamma, beta):
    mu = jnp.mean(x, axis=-1, keepdims=True)
    var = jnp.mean(jnp.square(x - mu), axis=-1, keepdims=True)
    return (x - mu) * jax.lax.rsqrt(var + LN_EPS) * gamma + beta


def _shard_fn(h_sh, ei0, ei1, e2g, fd, lat9, ln_gamma, ln_beta,
              eW1, eb1, eW2, eb2, nW1, nb1, nW2, nb2):
    # h_sh [NS,H] node shard; ei* [Eloc]; e2g [Eloc]; fd [Eloc,3]; lat9 [G,9]
    h = jax.lax.all_gather(h_sh, 'x', axis=0, tiled=True)   # [N,H]
    h_ln = _layernorm(h, ln_gamma, ln_beta)
    hi = h_ln[ei0]                        # [Eloc,H]
    hj = h_ln[ei1]
    lat_e = lat9[e2g]                     # [Eloc,9]
    freqs = 2.0 * np.pi * jnp.arange(NF, dtype=fd.dtype)
    emb = (fd[:, :, None] * freqs[None, None, :]).reshape(-1, 3 * NF)
    fe = jnp.concatenate([jnp.sin(emb), jnp.cos(emb)], axis=-1)  # [Eloc,384]
    e = jnp.concatenate([hi, hj, lat_e, fe], axis=1)             # [Eloc,905]
    e = jax.nn.silu(e @ eW1 + eb1)
    e = jax.nn.silu(e @ eW2 + eb2)                               # [Eloc,H]
    seg = ei0
    s = jax.ops.segment_sum(e, seg, num_segments=N)              # [N,H]
    c = jax.ops.segment_sum(jnp.ones((e.shape[0],), e.dtype), seg,
                            num_segments=N)                      # [N]
    s = jax.lax.psum_scatter(s, 'x', scatter_dimension=0, tiled=True)  # [NS,H]
    c = jax.lax.psum_scatter(c, 'x', scatter_dimension=0, tiled=True)  # [NS]
    agg = s / jnp.maximum(c, 1.0)[:, None]
    h_ln_sh = _layernorm(h_sh, ln_gamma, ln_beta)
    out = jnp.concatenate([h_ln_sh, agg], axis=1)                # [NS,2H]
    out = jax.nn.silu(out @ nW1 + nb1)
    out = jax.nn.silu(out @ nW2 + nb2)
    return h_sh + out                                            # [NS,H]


def _get_jit():
    if 'fn' in _cache:
        return _cache['fn'], _cache['mesh']
    mesh = jax.make_mesh((M,), ('x',))
    rep = P()
    fn = jax.jit(jax.shard_map(
        _shard_fn, mesh=mesh,
        in_specs=(P('x', None), P('x'), P('x'), P('x'), P('x', None), rep,
                  rep, rep, rep, rep, rep, rep, rep, rep, rep, rep),
        out_specs=P('x', None)))
    _cache['fn'] = fn
    _cache['mesh'] = mesh
    return fn, mesh


def kernel(h, frac_coords, lattices, edge_index, edge2graph, frac_diff,
           ln_gamma, ln_beta, eW1, eb1, eW2, eb2, nW1, nb1, nW2, nb2):
    fn, mesh = _get_jit()
    lat = np.asarray(lattices, np.float32)
    lat9 = np.einsum('gij,gkj->gik', lat, lat).reshape(G, 9)
    ei = np.asarray(edge_index, np.int32)
    ei0 = np.ascontiguousarray(ei[0]); ei1 = np.ascontiguousarray(ei[1])
    e2g = np.asarray(edge2graph, np.int32)
    args = (np.asarray(h, np.float32), ei0, ei1, e2g,
            np.asarray(frac_diff, np.float32), lat9.astype(np.float32),
            np.asarray(ln_gamma, np.float32), np.asarray(ln_beta, np.float32),
            np.asarray(eW1, np.float32), np.asarray(eb1, np.float32),
            np.asarray(eW2, np.float32), np.asarray(eb2, np.float32),
            np.asarray(nW1, np.float32), np.asarray(nb1, np.float32),
            np.asarray(nW2, np.float32), np.asarray(nb2, np.float32))
    import time
    from jax.sharding import NamedSharding
    specs = (P('x', None), P('x'), P('x'), P('x'), P('x', None), P(),
             P(), P(), P(), P(), P(), P(), P(), P(), P(), P())
    t0 = time.perf_counter()
    dargs = [jax.device_put(a, NamedSharding(mesh, s))
             for a, s in zip(args, specs)]
    for a in dargs:
        a.block_until_ready()
    t1 = time.perf_counter()
    out = fn(*dargs)
    out.block_until_ready()
    t2 = time.perf_counter()
    res = np.asarray(jax.device_get(out), np.float32)
    t3 = time.perf_counter()
    _timing.update(h2d=round(t1-t0,3), exec=round(t2-t1,3), d2h=round(t3-t2,3))
    return res
